# revision 8
# baseline (speedup 1.0000x reference)
"""Trainium2 Bass kernel for nn_David_46523085750370 (sparse_attention).

Strategy: pure data-parallel over batch (8192 rows -> 8 cores x 1024).
Per core, rows are processed in 2 superblocks of 512 rows (4 chunks of 128).

Math pipeline per row (see reference):
  common_i = sf_i @ Ws_i.T + (bs_i + scale_emb_i)                 [5 x 1024]
  Q  = feat @ Wq.T + bq;  K_i = common_i @ Wk.T + bk;  V_i likewise
  dots[h,i] = <Q_h, K_i_h>;  scores over static cantor routes; softmax(3)
  out[h,s] = sum_w attn * V_route;  si[s] = mean_h ||out[h,s]||
  gate: LN(feat @ Wg1.T + bg1) -> gelu_exact -> @ Wg2.T + bg2
  aw = softmax(0.7*si + 0.3*gl);  logits = sum_s aw_s * logits_s

Device mapping:
  - All matmuls in bf16 inputs / fp32 PSUM accumulation (empirically ~2e-3
    max rel err end-to-end, measured against the fp32 reference in numpy).
  - commonT produced directly transposed (PE Option B) so it can serve as
    the stationary operand of the K/V matmuls without on-device transposes.
    Activations are transposed on the host (part of sharding prep).
  - common/K/V/Q/gate biases folded: common bias added during PSUM->SBUF
    evacuation (per-partition scalar on ACT); K/V/Q/gate biases added with a
    rank-1 ones-row matmul accumulated into PSUM.
  - attention gather over scales is static (routes is a deterministic
    constant); softmax over 3 elems without max subtraction (|score| < ~50).
  - engine split: PE matmuls; ACT evacuations + exp/erf/sqrt/square;
    DVE dots/softmax/norms/gate/logit-combine; GPSIMD part of the AV combine.
"""

import os

import numpy as np
import ml_dtypes

import concourse.bass as bass
import concourse.mybir as mybir
import concourse.tile as tile
import bass_rust
from concourse.bass_utils import run_bass_kernel_spmd

F32 = mybir.dt.float32
BF16 = mybir.dt.bfloat16
AF = mybir.ActivationFunctionType
ALU = mybir.AluOpType

NCORES = 8
B = 8192
FD = 1024
H = 16
D = 64
S = 5
LW = 3
C = 1000
GD = 512
SCALES = [64, 128, 256, 512, 1024]
# cantor routes (deterministic, matches reference._build_routes())
ROUTES = [[0, 2, 1], [1, 0, 2], [0, 2, 1], [3, 1, 4], [4, 3, 1]]

_LAST_PERF = {}


def split_excess_waits(nc, max_waits=1):
    """walrus gen3 codegen only supports one sync-wait per instruction; hoist
    extra waits onto preceding wait-only Drains on the same engine."""
    nsplit = 0
    for f in nc.m.functions:
        for bb in f.blocks:
            insts = list(bb.instructions)
            changed = False
            new_insts = []
            for inst in insts:
                si = inst.sync_info
                if si is not None and si.on_wait and len(si.on_wait) > max_waits:
                    waits = list(si.on_wait)
                    chunks = [
                        waits[k : k + max_waits]
                        for k in range(0, len(waits), max_waits)
                    ]
                    for j, ch in enumerate(chunks[:-1]):
                        d = mybir.InstDrain(
                            name=f"{inst.name}-wsplit{j}",
                            opcode="Drain",
                            engine=inst.engine,
                            debug=inst.debug,
                            ins=[],
                            outs=[],
                            descendants=bass_rust.InstructionNameOrderedSet([]),
                            sync_info=mybir.SyncInfo(on_wait=list(ch), on_update=[]),
                        )
                        new_insts.append(d)
                        nsplit += 1
                    si.on_wait = chunks[-1]
                    inst.sync_info = si
                    changed = True
                new_insts.append(inst)
            if changed:
                bb.instructions = new_insts
    return nsplit


def pbcast(ap, p=128):
    """Broadcast a DRAM AP across p partitions (step-0 partition dim)."""
    return bass.AP(ap.tensor, ap.offset, [[0, p]] + list(ap.ap))


def build_nc(n_rows, sb_rows, inv_t, bg2eff, si_coef):
    """Emit the per-core kernel. n_rows rows, superblocks of sb_rows."""
    nc = bass.Bass()

    # ---- DRAM I/O (per-core shapes) ----
    featT = nc.dram_tensor("featT", [FD, n_rows], BF16, kind="ExternalInput")
    sfT = [
        nc.dram_tensor(f"sfT_{i}", [SCALES[i], n_rows], BF16, kind="ExternalInput")
        for i in range(S)
    ]
    lg_in = [
        nc.dram_tensor(f"logits_{i}", [n_rows, C], F32, kind="ExternalInput")
        for i in range(S)
    ]
    wsT = [
        nc.dram_tensor(f"wsT_{i}", [SCALES[i], FD], BF16, kind="ExternalInput")
        for i in range(S)
    ]
    wqT = nc.dram_tensor("wqT", [FD, FD], BF16, kind="ExternalInput")
    wkT = nc.dram_tensor("wkT", [FD, FD], BF16, kind="ExternalInput")
    wvT = nc.dram_tensor("wvT", [FD, FD], BF16, kind="ExternalInput")
    wg1T = nc.dram_tensor("wg1T", [FD, GD], BF16, kind="ExternalInput")
    # rows: 0=bq, 1=bg1(padded), 2+i=kbias_i, 7+i=vbias_i
    biasrows = nc.dram_tensor("biasrows", [1, 12 * FD], BF16, kind="ExternalInput")
    cbT = nc.dram_tensor("cbT", [128, S * (FD // 128)], F32, kind="ExternalInput")
    gamma_in = nc.dram_tensor("gamma", [GD], F32, kind="ExternalInput")
    beta_in = nc.dram_tensor("beta", [GD], F32, kind="ExternalInput")
    wg2e = nc.dram_tensor("wg2e", [S, GD], BF16, kind="ExternalInput")
    bg2_in = nc.dram_tensor("bg2e", [S], F32, kind="ExternalInput")
    out_logits = nc.dram_tensor("out_logits", [n_rows, C], F32, kind="ExternalOutput")
    out_aw = nc.dram_tensor("out_aw", [n_rows, S], F32, kind="ExternalOutput")

    KB = FD // 128  # contraction blocks over the 1024 feature dim
    MB = FD // 128  # common-feature blocks
    n_sb = n_rows // sb_rows
    n_ch = sb_rows // 128

    with tile.TileContext(nc) as tc:
        import contextlib

        ctx = contextlib.ExitStack()
        with ctx:
            const = ctx.enter_context(tc.tile_pool(name="const", bufs=1))
            work = ctx.enter_context(tc.tile_pool(name="work", bufs=2))
            psum = ctx.enter_context(tc.tile_pool(name="psum", bufs=1, space="PSUM"))

            # ---- constants / resident weights ----
            wq_t = const.tile([128, KB, FD], BF16, name="wq_t", tag="wq_t")
            nc.sync.dma_start(out=wq_t, in_=wqT[:, :].rearrange("(k p) m -> p k m", p=128))
            wk_t = const.tile([128, KB, FD], BF16, name="wk_t", tag="wk_t")
            nc.sync.dma_start(out=wk_t, in_=wkT[:, :].rearrange("(k p) m -> p k m", p=128))
            wv_t = const.tile([128, KB, FD], BF16, name="wv_t", tag="wv_t")
            nc.sync.dma_start(out=wv_t, in_=wvT[:, :].rearrange("(k p) m -> p k m", p=128))
            wg1_t = const.tile([128, KB, GD], BF16, name="wg1_t", tag="wg1_t")
            nc.sync.dma_start(out=wg1_t, in_=wg1T[:, :].rearrange("(k p) m -> p k m", p=128))
            cb_t = const.tile([128, S * MB], F32, name="cb_t", tag="cb_t")
            nc.sync.dma_start(out=cb_t, in_=cbT[:, :])
            gam_t = const.tile([128, GD], F32, name="gam_t", tag="gam_t")
            nc.sync.dma_start(out=gam_t, in_=pbcast(gamma_in[:]))
            bet_t = const.tile([128, GD], F32, name="bet_t", tag="bet_t")
            nc.sync.dma_start(out=bet_t, in_=pbcast(beta_in[:]))
            wg2_t = const.tile([128, S, GD], BF16, name="wg2_t", tag="wg2_t")
            nc.sync.dma_start(out=wg2_t, in_=pbcast(wg2e[:, :]))
            br_t = const.tile([1, 12 * FD], BF16, name="br_t", tag="br_t")
            nc.sync.dma_start(out=br_t, in_=biasrows[:, :])

            def br(row, lo, hi):
                return br_t[0:1, row * FD + lo : row * FD + hi]
            bg2_t = const.tile([128, S], F32, name="bg2_t", tag="bg2_t")
            nc.sync.dma_start(out=bg2_t, in_=pbcast(bg2_in[:]))
            ones_t = const.tile([1, 128], BF16, name="ones_t", tag="ones_t")
            nc.vector.memset(ones_t, 1.0)
            eps_t = const.tile([128, 1], F32, name="eps_t", tag="eps_t")
            nc.vector.memset(eps_t, 1e-5)

            for sb in range(n_sb):
                r0 = sb * sb_rows

                # ================= Phase 1: commonT per scale =================
                # ct_i: [128 cfeat-in-block, MB blocks, sb_rows], bf16
                ct = []
                for i in range(S):
                    t = work.tile(
                        [128, MB, sb_rows], BF16,
                        name=f"ct{i}_{sb}", tag=f"ct{i}", bufs=1,
                    )
                    ct.append(t)
                for i in range(S):
                    nkb = (SCALES[i] + 127) // 128
                    for mbp in range(MB // 2):  # pairs of output blocks
                        pcs = []
                        for half in range(2):
                            p = psum.tile(
                                [128, sb_rows], F32,
                                name=f"psc_{sb}_{i}_{mbp}_{half}",
                                tag="half", bufs=2,
                            )
                            pcs.append(p)
                        for kb in range(nkb):
                            kpart = min(128, SCALES[i] - kb * 128)
                            wsb = work.tile(
                                [128, 256], BF16,
                                name=f"wsb_{sb}_{i}_{mbp}_{kb}", tag="wsb", bufs=3,
                            )
                            nc.sync.dma_start(
                                out=wsb[:kpart, :],
                                in_=wsT[i][
                                    kb * 128 : kb * 128 + kpart,
                                    mbp * 256 : (mbp + 1) * 256,
                                ],
                            )
                            sfb = work.tile(
                                [128, sb_rows], BF16,
                                name=f"sfb_{sb}_{i}_{mbp}_{kb}", tag="sfb", bufs=3,
                            )
                            nc.sync.dma_start(
                                out=sfb[:kpart, :],
                                in_=sfT[i][kb * 128 : kb * 128 + kpart, r0 : r0 + sb_rows],
                            )
                            for half in range(2):
                                nc.tensor.matmul(
                                    pcs[half][:, :],
                                    wsb[:kpart, half * 128 : half * 128 + 128],
                                    sfb[:kpart, :],
                                    start=(kb == 0),
                                    stop=(kb == nkb - 1),
                                )
                        for half in range(2):
                            mb = mbp * 2 + half
                            nc.scalar.activation(
                                out=ct[i][:, mb, :],
                                in_=pcs[half][:, :],
                                func=AF.Identity,
                                bias=cb_t[:, i * MB + mb : i * MB + mb + 1],
                                scale=1.0,
                            )

                # feat blocks for this superblock: [128, KB, sb_rows]
                ft = work.tile(
                    [128, KB, sb_rows], BF16, name=f"ft_{sb}", tag="ft", bufs=1
                )
                nc.sync.dma_start(
                    out=ft,
                    in_=featT[:, :].rearrange("(k p) r -> p k r", p=128)[
                        :, :, r0 : r0 + sb_rows
                    ],
                )

                # ================= Phase 2+3: per chunk =================
                for c in range(n_ch):
                    rows = slice(r0 + c * 128, r0 + (c + 1) * 128)
                    cs = slice(c * 128, (c + 1) * 128)

                    # ---- Q and gate h1 matmuls ----
                    ps_q = psum.tile(
                        [128, FD], F32, name=f"psq_{sb}_{c}", tag="full", bufs=3
                    )
                    ps_h = psum.tile(
                        [128, GD], F32, name=f"psh_{sb}_{c}", tag="half", bufs=2
                    )
                    for kb in range(KB):
                        lhs = ft[:, kb, cs]
                        for half in range(2):
                            nc.tensor.matmul(
                                ps_q[:, half * 512 : half * 512 + 512],
                                lhs,
                                wq_t[:, kb, half * 512 : half * 512 + 512],
                                start=(kb == 0),
                                stop=False,
                            )
                        nc.tensor.matmul(
                            ps_h[:, :], lhs, wg1_t[:, kb, :],
                            start=(kb == 0), stop=False,
                        )
                    for half in range(2):
                        nc.tensor.matmul(
                            ps_q[:, half * 512 : half * 512 + 512],
                            ones_t[:, :],
                            br(0, half * 512, half * 512 + 512),
                            start=False, stop=True,
                        )
                    nc.tensor.matmul(
                        ps_h[:, :], ones_t[:, :], br(1, 0, GD),
                        start=False, stop=True,
                    )
                    q_t = work.tile(
                        [128, FD], BF16, name=f"q_{sb}_{c}", tag="q", bufs=3
                    )
                    nc.scalar.copy(out=q_t, in_=ps_q)

                    # ---- gate: LN -> gelu(erf) -> gate logits ----
                    stats = work.tile(
                        [128, 6], F32, name=f"gst_{sb}_{c}", tag="gst", bufs=2
                    )
                    nc.vector.bn_stats(out=stats, in_=ps_h[:, :])
                    mv = work.tile(
                        [128, 2], F32, name=f"gmv_{sb}_{c}", tag="gmv", bufs=2
                    )
                    nc.vector.bn_aggr(out=mv, in_=stats)
                    sd = work.tile(
                        [128, 1], F32, name=f"gsd_{sb}_{c}", tag="gsd", bufs=2
                    )
                    nc.scalar.activation(
                        out=sd, in_=mv[:, 1:2], func=AF.Sqrt, bias=eps_t, scale=1.0
                    )
                    rstd = work.tile(
                        [128, 1], F32, name=f"grs_{sb}_{c}", tag="grs", bufs=2
                    )
                    nc.vector.reciprocal(out=rstd, in_=sd)
                    hn = work.tile(
                        [128, GD], F32, name=f"ghn_{sb}_{c}", tag="ghn", bufs=1
                    )
                    nc.vector.tensor_scalar(
                        out=hn, in0=ps_h[:, :],
                        scalar1=mv[:, 0:1], scalar2=rstd,
                        op0=ALU.subtract, op1=ALU.mult,
                    )
                    ha = work.tile(
                        [128, GD], F32, name=f"gha_{sb}_{c}", tag="gha", bufs=1
                    )
                    nc.vector.tensor_mul(ha, hn, gam_t)
                    nc.vector.tensor_add(ha, ha, bet_t)
                    ef = work.tile(
                        [128, GD], F32, name=f"gef_{sb}_{c}", tag="gef", bufs=1
                    )
                    nc.scalar.activation(
                        out=ef, in_=ha, func=AF.Erf, scale=float(1.0 / np.sqrt(2.0))
                    )
                    ga = work.tile(
                        [128, GD], F32, name=f"gga_{sb}_{c}", tag="gga", bufs=1
                    )
                    # x*(1+erf(x/sqrt2)); the 0.5 of gelu is folded into wg2e
                    nc.vector.scalar_tensor_tensor(
                        out=ga, in0=ef, scalar=1.0, in1=ha,
                        op0=ALU.add, op1=ALU.mult,
                    )
                    gl_t = work.tile(
                        [128, S], F32, name=f"gl_{sb}_{c}", tag="gl", bufs=3
                    )
                    ttr_s = work.tile(
                        [128, GD], F32, name=f"ttr_{sb}_{c}", tag="ttr", bufs=1
                    )
                    for s in range(S):
                        nc.vector.tensor_mul(ttr_s, ga, wg2_t[:, s, :])
                        nc.vector.reduce_sum(
                            out=gl_t[:, s : s + 1],
                            in_=ttr_s.rearrange("p (o g) -> p o g", o=1),
                            axis=mybir.AxisListType.X,
                        )
                    nc.vector.tensor_add(gl_t, gl_t, bg2_t)

                    # ---- K/V per scale + dots ----
                    dots = work.tile(
                        [128, H, S], F32, name=f"dots_{sb}_{c}", tag="dots", bufs=2
                    )
                    v_t = []
                    for i in range(S):
                        ps_k = psum.tile(
                            [128, FD], F32, name=f"psk_{sb}_{c}_{i}",
                            tag="full", bufs=3,
                        )
                        ps_v = psum.tile(
                            [128, FD], F32, name=f"psv_{sb}_{c}_{i}",
                            tag="full", bufs=3,
                        )
                        for kb in range(KB):
                            lhs = ct[i][:, kb, cs]
                            for half in range(2):
                                hs = slice(half * 512, half * 512 + 512)
                                nc.tensor.matmul(
                                    ps_k[:, hs], lhs, wk_t[:, kb, hs],
                                    start=(kb == 0), stop=False,
                                    skip_group_check=True,
                                )
                                nc.tensor.matmul(
                                    ps_v[:, hs], lhs, wv_t[:, kb, hs],
                                    start=(kb == 0), stop=False,
                                    skip_group_check=True,
                                )
                        for half in range(2):
                            hs = slice(half * 512, half * 512 + 512)
                            nc.tensor.matmul(
                                ps_k[:, hs], ones_t[:, :], br(2 + i, half * 512, half * 512 + 512),
                                start=False, stop=True, skip_group_check=True,
                            )
                            nc.tensor.matmul(
                                ps_v[:, hs], ones_t[:, :], br(7 + i, half * 512, half * 512 + 512),
                                start=False, stop=True, skip_group_check=True,
                            )
                        vt = work.tile(
                            [128, FD], BF16, name=f"v_{sb}_{c}_{i}", tag="v", bufs=5
                        )
                        nc.scalar.copy(out=vt, in_=ps_v)
                        v_t.append(vt)
                        qk = work.tile(
                            [128, FD], F32, name=f"qk_{sb}_{c}_{i}", tag="qk", bufs=1
                        )
                        nc.vector.tensor_mul(qk, q_t, ps_k[:, :])
                        nc.vector.reduce_sum(
                            out=dots[:, :, i],
                            in_=qk.rearrange("p (h d) -> p h d", d=D),
                            axis=mybir.AxisListType.X,
                        )

                    # ---- softmax over routed windows (static routes) ----
                    ed = work.tile(
                        [128, H, S], F32, name=f"ed_{sb}_{c}", tag="ed", bufs=2
                    )
                    nc.scalar.activation(out=ed, in_=dots, func=AF.Exp, scale=inv_t)
                    den = work.tile(
                        [128, H, S], F32, name=f"den_{sb}_{c}", tag="den", bufs=2
                    )
                    for s in range(S):
                        r = ROUTES[s]
                        nc.vector.tensor_add(
                            den[:, :, s], ed[:, :, r[0]], ed[:, :, r[1]]
                        )
                        nc.vector.tensor_add(
                            den[:, :, s], den[:, :, s], ed[:, :, r[2]]
                        )
                    rden = work.tile(
                        [128, H, S], F32, name=f"rden_{sb}_{c}", tag="rden", bufs=2
                    )
                    nc.vector.reciprocal(out=rden, in_=den)
                    attn = work.tile(
                        [128, H, S, LW], F32, name=f"attn_{sb}_{c}", tag="attn", bufs=2
                    )
                    for s in range(S):
                        for w in range(LW):
                            nc.vector.tensor_mul(
                                attn[:, :, s, w], ed[:, :, ROUTES[s][w]], rden[:, :, s]
                            )

                    # ---- out_s = sum_w attn * V_route; si = mean_h ||out|| ----
                    nrm = work.tile(
                        [128, H, S], F32, name=f"nrm_{sb}_{c}", tag="nrm", bufs=2
                    )
                    for s in range(S):
                        eng = nc.vector if s < 3 else nc.gpsimd
                        av = work.tile(
                            [128, H, D], F32, name=f"av_{sb}_{c}_{s}", tag="av", bufs=1
                        )
                        tmp = work.tile(
                            [128, H, D], F32, name=f"avt_{sb}_{c}_{s}", tag="avt",
                            bufs=1,
                        )
                        for w in range(LW):
                            vr = v_t[ROUTES[s][w]].rearrange("p (h d) -> p h d", d=D)
                            ab = attn[:, :, s, w].unsqueeze(2).broadcast_to(
                                (128, H, D)
                            )
                            if w == 0:
                                eng.tensor_mul(av, vr, ab)
                            else:
                                eng.tensor_mul(tmp, vr, ab)
                                eng.tensor_add(av, av, tmp)
                        sq = work.tile(
                            [128, H, D], F32, name=f"sq_{sb}_{c}_{s}", tag="sq", bufs=1
                        )
                        nc.scalar.activation(out=sq, in_=av, func=AF.Square)
                        nc.vector.reduce_sum(
                            out=nrm[:, :, s], in_=sq, axis=mybir.AxisListType.X
                        )
                    srt = work.tile(
                        [128, H, S], F32, name=f"srt_{sb}_{c}", tag="srt", bufs=2
                    )
                    nc.scalar.activation(out=srt, in_=nrm, func=AF.Sqrt)
                    sisum = work.tile(
                        [128, S], F32, name=f"sis_{sb}_{c}", tag="sis", bufs=2
                    )
                    nc.vector.reduce_sum(
                        out=sisum,
                        in_=srt.rearrange("p h s -> p s h"),
                        axis=mybir.AxisListType.X,
                    )

                    # ---- combined scores -> aw ----
                    cmb = work.tile(
                        [128, S], F32, name=f"cmb_{sb}_{c}", tag="cmb", bufs=2
                    )
                    nc.vector.scalar_tensor_tensor(
                        out=cmb, in0=sisum, scalar=float(si_coef), in1=gl_t,
                        op0=ALU.mult, op1=ALU.add,
                    )
                    e5 = work.tile([128, S], F32, name=f"e5_{sb}_{c}", tag="e5", bufs=2)
                    nc.scalar.activation(out=e5, in_=cmb, func=AF.Exp)
                    s5 = work.tile([128, 1], F32, name=f"s5_{sb}_{c}", tag="s5", bufs=2)
                    nc.vector.reduce_sum(out=s5, in_=e5, axis=mybir.AxisListType.X)
                    r5 = work.tile([128, 1], F32, name=f"r5_{sb}_{c}", tag="r5", bufs=2)
                    nc.vector.reciprocal(out=r5, in_=s5)
                    aw_t = work.tile(
                        [128, S], F32, name=f"aw_{sb}_{c}", tag="aw", bufs=2
                    )
                    nc.vector.tensor_scalar_mul(out=aw_t, in0=e5, scalar1=r5)
                    nc.sync.dma_start(out=out_aw[rows, :], in_=aw_t)

                    # ---- weighted combine of class logits ----
                    acc_a = work.tile(
                        [128, C], F32, name=f"acca_{sb}_{c}", tag="acca", bufs=1
                    )
                    acc_b = work.tile(
                        [128, C], F32, name=f"accb_{sb}_{c}", tag="accb", bufs=1
                    )
                    accs = [acc_a, acc_b]
                    prev = None
                    for s in range(S):
                        lg = work.tile(
                            [128, C], F32, name=f"lg_{sb}_{c}_{s}", tag="lg", bufs=2
                        )
                        nc.sync.dma_start(out=lg, in_=lg_in[s][rows, :])
                        dst = accs[s % 2]
                        if s == 0:
                            nc.vector.tensor_scalar_mul(
                                out=dst, in0=lg, scalar1=aw_t[:, 0:1]
                            )
                        else:
                            nc.vector.scalar_tensor_tensor(
                                out=dst, in0=lg, scalar=aw_t[:, s : s + 1],
                                in1=prev, op0=ALU.mult, op1=ALU.add,
                            )
                        prev = dst
                    nc.sync.dma_start(out=out_logits[rows, :], in_=prev)

    split_excess_waits(nc)
    return nc


def _prep(inputs, n_rows):
    """Host-side: fold biases, transpose + bf16-ify, build per-core maps."""
    bf = ml_dtypes.bfloat16
    f32 = np.float32
    g = {k: np.asarray(v) for k, v in inputs.items()}

    temperature = float(np.abs(g["temperature"]))
    inv_t = float(1.0 / (np.sqrt(D) * temperature))
    cb = (g["bs"] + g["scale_emb"]).astype(f32)  # [S, FD]
    # cb is added into commonT at evacuation, so K/V rows carry only bk/bv
    kbias = np.broadcast_to(g["bk"].astype(f32), (S, FD))
    vbias = np.broadcast_to(g["bv"].astype(f32), (S, FD))
    bg2eff = (0.3 * g["bg2"]).astype(f32)
    wg2eff = (0.15 * g["Wg2"]).astype(bf)  # 0.3 gate coef * 0.5 gelu
    si_coef = 0.7 / H

    biasrows = np.zeros((12, FD), f32)
    biasrows[0] = g["bq"]
    biasrows[1, :GD] = g["bg1"]
    biasrows[2:7] = kbias
    biasrows[7:12] = vbias
    biasrows = biasrows.astype(bf).reshape(1, -1)

    cbT = np.ascontiguousarray(
        cb.reshape(S, FD // 128, 128).transpose(2, 0, 1).reshape(128, S * (FD // 128))
    ).astype(f32)

    shared = {
        "wqT": np.ascontiguousarray(g["Wq"].T).astype(bf),
        "wkT": np.ascontiguousarray(g["Wk"].T).astype(bf),
        "wvT": np.ascontiguousarray(g["Wv"].T).astype(bf),
        "wg1T": np.ascontiguousarray(g["Wg1"].T).astype(bf),
        "biasrows": biasrows,
        "cbT": cbT,
        "gamma": g["g_gamma"].astype(f32),
        "beta": g["g_beta"].astype(f32),
        "wg2e": wg2eff,
        "bg2e": bg2eff.astype(f32),
    }
    for i in range(S):
        shared[f"wsT_{i}"] = np.ascontiguousarray(g[f"Ws_{i}"].T).astype(bf)

    featT = np.ascontiguousarray(g["features"].T).astype(bf)  # [FD, B]
    sfTs = [np.ascontiguousarray(g[f"sf_{i}"].T).astype(bf) for i in range(S)]

    in_maps = []
    ncores = g["features"].shape[0] // n_rows
    for cid in range(ncores):
        r = slice(cid * n_rows, (cid + 1) * n_rows)
        m = dict(shared)
        m["featT"] = np.ascontiguousarray(featT[:, r])
        for i in range(S):
            m[f"sfT_{i}"] = np.ascontiguousarray(sfTs[i][:, r])
            m[f"logits_{i}"] = np.ascontiguousarray(g[f"logits_{i}"][r]).astype(f32)
        in_maps.append(m)
    return in_maps, inv_t, bg2eff, si_coef


def kernel(**inputs):
    n_rows = int(os.environ.get("KERNEL_ROWS", "1024"))
    sb_rows = int(os.environ.get("KERNEL_SB_ROWS", "512"))
    in_maps, inv_t, bg2eff, si_coef = _prep(inputs, n_rows)
    ncores = len(in_maps)

    nc = build_nc(n_rows, sb_rows, inv_t, bg2eff, si_coef)
    res = run_bass_kernel_spmd(nc, in_maps, core_ids=list(range(ncores)))

    _LAST_PERF.clear()
    _LAST_PERF.update(
        exec_time_ns=res.exec_time_ns,
        mean_exec_time_ns=res.mean_exec_time_ns,
        trace=res.instructions_and_trace[1] if res.instructions_and_trace else None,
    )
    if res.exec_time_ns is not None:
        print(f"HW exec time: {res.exec_time_ns} ns")

    logits = np.concatenate([r["out_logits"] for r in res.results], axis=0)
    aw = np.concatenate([r["out_aw"] for r in res.results], axis=0)
    return logits, aw


if __name__ == "__main__":
    rng = np.random.default_rng(0)
    fake = {}
    Bn = int(os.environ.get("KERNEL_ROWS", "1024")) * 8
    fake["features"] = rng.standard_normal((Bn, FD), dtype=np.float32)
    for i, sc in enumerate(SCALES):
        fake[f"sf_{i}"] = rng.standard_normal((Bn, sc), dtype=np.float32)
        fake[f"Ws_{i}"] = rng.standard_normal((FD, sc), dtype=np.float32) * 0.02
        fake[f"logits_{i}"] = rng.standard_normal((Bn, C), dtype=np.float32)
    fake["bs"] = np.zeros((S, FD), np.float32)
    fake["scale_emb"] = rng.standard_normal((S, FD), dtype=np.float32) * 0.02
    for w, shp in [("Wq", (FD, FD)), ("Wk", (FD, FD)), ("Wv", (FD, FD)),
                   ("Wg1", (GD, FD)), ("Wg2", (S, GD))]:
        fake[w] = rng.standard_normal(shp, dtype=np.float32) * 0.02
    for b, n in [("bq", FD), ("bk", FD), ("bv", FD), ("bg1", GD), ("bg2", S)]:
        fake[b] = np.zeros((n,), np.float32)
    fake["g_gamma"] = np.ones((GD,), np.float32)
    fake["g_beta"] = np.zeros((GD,), np.float32)
    fake["temperature"] = np.float32(0.07)
    fake["routes"] = np.asarray(ROUTES, np.int32)
    out = kernel(**fake)
    print("ran:", out[0].shape, out[1].shape)
